# revision 38
# baseline (speedup 1.0000x reference)
"""YOLO-style detector decode kernel for Trainium2 (8 NeuronCores, SPMD).

Sharding: pure data parallel over the batch dim (128 -> 16 per core).

Layout: for each head (13/26/52) the host prepacks the 45 channels
(3 anchors x 15 roles) into [G, B, 3, nchunk, 15, FC] so that each SBUF
partition row (g, b, a) loads its whole chunk (15 roles x FC cells) as ONE
contiguous ~40KB DMA descriptor -- real TRN2 DMA is descriptor-rate-bound
(~38ns/descriptor), so small per-role descriptors are 6x slower than the
HBM roofline.  Outputs are likewise packed as [G, B, 3, nchunk, 7, FC]
(iou, x1, y1, x2, y2, kind, mask) and unpacked on the host with cheap numpy
transposes.

Decode per cell-anchor (partition p = g*48 + b*3 + a, free = cells):

  pre   = grid*stride + stride*d        (gpsimd iota grid + one fused DVE STT)
  half  = exp(dw + ln(anchor/2))        (single ACT op; bias folds the mul,
                                         ACT only ever runs Exp -> no
                                         activation-table reloads)
  x1/x2 = pre -/+ half                  (DVE)
  mask  = iou > thresh                  (gpsimd, off the DVE critical path)
  kind  = tournament argmax over the 10 class planes (DVE)
"""

import numpy as np

import concourse.bass as bass
import concourse.bacc as bacc
import concourse.mybir as mybir
from concourse.tile import TileContext
from concourse.bass_utils import run_bass_kernel_spmd

F32 = mybir.dt.float32
U8 = mybir.dt.uint8
ALU = mybir.AluOpType
ACTF = mybir.ActivationFunctionType

NCORES = 8
B = 128
BL = B // NCORES  # 16

# name, H(=W), stride, G (cell-space split to widen partitions), n free chunks
HEADS = [
    ("13", 13, 32, 1, 1),
    ("26", 26, 16, 2, 1),
    ("52", 52, 8, 2, 2),
]


def _build_nc(reps=1, variant="full"):
    # Bacc (not raw Bass): its compile() legalizes multi-wait sync_info into
    # event semaphores (hardware allows one wait per instruction).
    # reps>1 wraps the body in a For_i loop -- used only for benchmarking
    # (fixed host/proxy overhead cancels between reps=1 and reps=R runs).
    # variant: "full" | "dma" (no compute) | "compute" (no chunk DMAs).
    import contextlib
    nc = bacc.Bacc("TRN2", target_bir_lowering=False)
    ins = {}
    outs = {}
    for name, H, S, G, NCH in HEADS:
        HW = H * H
        FC = HW // (G * NCH)
        ins[name] = nc.declare_dram_parameter(
            f"in{name}", [G, BL, 3, NCH, 15, FC], F32, isOutput=False)
        outs[name] = nc.declare_dram_parameter(
            f"out{name}", [G, BL, 3, NCH, 7, FC], F32, isOutput=True)
    consts = nc.declare_dram_parameter("consts", [96, 8], F32, isOutput=False)

    with TileContext(nc) as tc:
        with (
            tc.tile_pool(name="persist", bufs=1) as ppool,
            tc.tile_pool(name="work", bufs=2) as wpool,
            tc.tile_pool(name="scratch", bufs=1) as spool,
        ):
            consts_t = ppool.tile([96, 8], F32, tag="consts")
            nc.sync.dma_start(out=consts_t[:, :], in_=consts[:])

            # Per-head grid tiles (stride-scaled cell coordinates), built once
            # on gpsimd via iota.  Values <= 408, exact in f32.
            grids = {}
            for name, H, S, G, NCH in HEADS:
                P = 48 * G
                H2 = H // G
                F = H2 * H
                gx = ppool.tile([P, F], F32, tag=f"gx{name}")
                gy = ppool.tile([P, F], F32, tag=f"gy{name}")
                nc.gpsimd.iota(
                    gx[:, :], pattern=[[0, H2], [S, H]], base=0,
                    channel_multiplier=0, allow_small_or_imprecise_dtypes=True,
                )
                # iota can't write at a partition offset (and partition windows
                # must be 32-aligned), so write the g=0 pattern everywhere and
                # add the g=1 half-offset (H2*S = 208 for both split heads)
                # from consts col 7 as a one-time per-partition scalar.
                nc.gpsimd.iota(
                    gy[:, :], pattern=[[S, H2], [0, H]], base=0,
                    channel_multiplier=0, allow_small_or_imprecise_dtypes=True,
                )
                if G == 2:
                    nc.vector.tensor_scalar(
                        out=gy[:, :], in0=gy[:, :],
                        scalar1=consts_t[0:P, 7:8], scalar2=None, op0=ALU.add,
                    )
                grids[name] = (gx, gy)

            rep_ctx = tc.For_i(0, reps, 1) if reps > 1 else contextlib.nullcontext()
            with rep_ctx:
                _emit_body(nc, tc, wpool, spool, consts_t, grids, ins, outs, variant)
    nc.compile()
    return nc


def _emit_body(nc, tc, wpool, spool, consts_t, grids, ins, outs, variant="full"):
    for hi, (name, H, S, G, NCH) in enumerate(HEADS):
        HW = H * H
        P = 48 * G
        F = HW // G
        FC = F // NCH
        gx, gy = grids[name]
        in_t = ins[name]
        out_t = outs[name]
        cw, ch = 2 * hi, 2 * hi + 1

        for c in range(NCH):
            cs = c * FC
            ce = cs + FC
            it = wpool.tile([P, 15 * FC], F32, tag="in")
            ot = wpool.tile([P, 7 * FC], F32, tag="out")
            if variant != "compute":
                # one ~(15*FC*4)B descriptor per partition row
                nc.sync.dma_start(out=it[:, :], in_=in_t[:, :, :, c, :, :])

            def sl(tile, r):
                return tile[:, r * FC:(r + 1) * FC]

            if variant == "dma":
                nc.gpsimd.tensor_copy(ot[:, 0:7 * FC], it[:, 0:7 * FC])
                nc.scalar.dma_start(out=out_t[:, :, :, c, :, :], in_=ot[:, :])
                continue

            hw2 = wpool.tile([P, FC], F32, tag="hw2")
            hh2 = wpool.tile([P, FC], F32, tag="hh2")
            # ACT runs only Exp (single function table, no reloads);
            # bias folds the anchor/2 multiplier: a/2*e^x = e^(x+ln(a/2))
            nc.scalar.activation(hw2[:, :], sl(it, 3), ACTF.Exp, bias=consts_t[0:P, cw:cw + 1])
            nc.scalar.activation(hh2[:, :], sl(it, 4), ACTF.Exp, bias=consts_t[0:P, ch:ch + 1])

            # pre = grid + S*d, fused STT, in place on the dx/dy role slices
            nc.vector.scalar_tensor_tensor(
                sl(it, 1), sl(it, 1), float(S), gx[:, cs:ce], ALU.mult, ALU.add)
            nc.vector.scalar_tensor_tensor(
                sl(it, 2), sl(it, 2), float(S), gy[:, cs:ce], ALU.mult, ALU.add)
            nc.vector.tensor_tensor(sl(ot, 1), sl(it, 1), hw2[:, :], ALU.subtract)
            nc.vector.tensor_tensor(sl(ot, 3), sl(it, 1), hw2[:, :], ALU.add)
            nc.vector.tensor_tensor(sl(ot, 2), sl(it, 2), hh2[:, :], ALU.subtract)
            nc.vector.tensor_tensor(sl(ot, 4), sl(it, 2), hh2[:, :], ALU.add)

            # iou passthrough + mask (gpsimd, off the DVE critical path)
            nc.gpsimd.tensor_copy(sl(ot, 0), sl(it, 0))
            nc.gpsimd.tensor_scalar(
                out=sl(ot, 6), in0=sl(it, 0),
                scalar1=consts_t[0:P, 6:7], scalar2=None, op0=ALU.is_gt)

            # tournament argmax over class role slices 5..14.  Pool (gpsimd)
            # has no tensor_tensor support in walrus codegen, so it can only
            # take the tensor_scalar idx adds; DVE keeps cmps/maxes/selects.
            cls = [sl(it, 5 + j) for j in range(10)]
            m = []   # running max (in place on even class slices)
            ix = []  # running argmax
            for i in range(5):
                # f32 gt: Pool ts-add can't mix dtypes (and only merges' g2
                # feed copy_predicated, which needs an integer mask)
                gt = spool.tile([P, FC], F32, tag=f"gt{i}")
                idx = spool.tile([P, FC], F32, tag=f"idx{i}")
                nc.vector.tensor_tensor(gt[:, :], cls[2 * i + 1], cls[2 * i], ALU.is_gt)
                nc.vector.tensor_tensor(cls[2 * i], cls[2 * i], cls[2 * i + 1], ALU.max)
                nc.gpsimd.tensor_scalar(
                    out=idx[:, :], in0=gt[:, :],
                    scalar1=float(2 * i), scalar2=None, op0=ALU.add)
                m.append(cls[2 * i])
                ix.append(idx[:, :])

            def merge(i, j):
                g2 = spool.tile([P, FC], U8, tag="gm")
                nc.vector.tensor_tensor(g2[:, :], m[j], m[i], ALU.is_gt)
                nc.vector.copy_predicated(ix[i], g2[:, :], ix[j])
                nc.vector.tensor_tensor(m[i], m[i], m[j], ALU.max)

            merge(0, 1)
            merge(2, 3)
            merge(0, 2)
            # final merge writes kind straight into the out tile
            gf = spool.tile([P, FC], U8, tag="gm")
            nc.vector.tensor_tensor(gf[:, :], m[4], m[0], ALU.is_gt)
            nc.vector.tensor_copy(sl(ot, 5), ix[0])
            nc.vector.copy_predicated(sl(ot, 5), gf[:, :], ix[4])

            if variant != "compute":
                nc.scalar.dma_start(out=out_t[:, :, :, c, :, :], in_=ot[:, :])


_NC_CACHE = {}


def _get_nc(reps=1, variant="full"):
    key = (reps, variant)
    if key not in _NC_CACHE:
        _NC_CACHE[key] = _build_nc(reps, variant)
    return _NC_CACHE[key]


def _host_inputs(output13, output26, output52, anchors13, anchors26, anchors52, thresh):
    consts = np.zeros((96, 8), np.float32)
    amod = np.arange(96) % 3
    for hi, anc in enumerate((anchors13, anchors26, anchors52)):
        anc = np.asarray(anc, np.float32)
        consts[:, 2 * hi] = np.log(anc[:, 0] / 2.0)[amod]
        consts[:, 2 * hi + 1] = np.log(anc[:, 1] / 2.0)[amod]
    consts[:, 6] = np.float32(thresh)
    consts[:, 7] = np.where(np.arange(96) >= 48, 208.0, 0.0)
    full = {"13": np.asarray(output13, np.float32),
            "26": np.asarray(output26, np.float32),
            "52": np.asarray(output52, np.float32)}
    # prepack: [B,45,H,W] -> (g, b, a, c, k, f) so each partition row's chunk
    # (15 roles x FC cells) is one contiguous DMA descriptor
    packed = {}
    for name, H, S, G, NCH in HEADS:
        HW = H * H
        FC = HW // (G * NCH)
        x = full[name].reshape(B, 3, 15, G, NCH, FC)  # (b, a, k, g, c, f)
        packed[name] = x.transpose(3, 0, 1, 4, 2, 5)  # (g, b, a, c, k, f)
    in_maps = []
    for cid in range(NCORES):
        mcore = {"consts": consts}
        for name, H, S, G, NCH in HEADS:
            sl = packed[name][:, cid * BL:(cid + 1) * BL]
            mcore[f"in{name}"] = np.ascontiguousarray(sl)
        in_maps.append(mcore)
    return in_maps


def _assemble(results):
    boxes_parts = []
    mask_parts = []
    for name, H, S, G, NCH in HEADS:
        HW = H * H
        FC = HW // (G * NCH)
        # [G, BL, 3, NCH, 7, FC] per core, concat batch
        ob = np.concatenate([r[f"out{name}"] for r in results], axis=1)
        # (g, b, a, c, r, f) -> (b, g, c, f, a, r) so (g,c,f) flattens to hw
        arr = ob.transpose(1, 0, 3, 5, 2, 4).reshape(B, HW, 3, 7)
        boxes_parts.append(arr[..., :6].reshape(-1, 6))
        mask_parts.append(arr[..., 6].reshape(-1) != 0)
    boxes = np.ascontiguousarray(np.concatenate(boxes_parts, 0))
    mask = np.concatenate(mask_parts, 0)
    return boxes, mask


def _run(trace=False, **inputs):
    nc = _get_nc()
    in_maps = _host_inputs(**inputs)
    res = run_bass_kernel_spmd(nc, in_maps, list(range(NCORES)), trace=trace)
    out = _assemble(res.results)
    return out, res


def kernel(**inputs):
    out, _ = _run(trace=False, **inputs)
    return out


def kernel_traced(**inputs):
    return _run(trace=True, **inputs)


# revision 39
# speedup vs baseline: 1.7242x; 1.7242x over previous
"""YOLO-style detector decode kernel for Trainium2 (8 NeuronCores, SPMD).

Sharding: pure data parallel over the batch dim (128 -> 16 per core).

Layout: for each head (13/26/52) the host prepacks the 45 channels
(3 anchors x 15 roles) into [G, B, 3, nchunk, 15, FC] so that each SBUF
partition row (g, b, a) loads its whole chunk (15 roles x FC cells) as ONE
contiguous ~40KB DMA descriptor -- real TRN2 DMA is descriptor-rate-bound
(~38ns/descriptor), so small per-role descriptors are 6x slower than the
HBM roofline.  Outputs are likewise packed as [G, B, 3, nchunk, 7, FC]
(iou, x1, y1, x2, y2, kind, mask) and unpacked on the host with cheap numpy
transposes.

Decode per cell-anchor (partition p = g*48 + b*3 + a, free = cells):

  pre   = grid*stride + stride*d        (gpsimd iota grid + one fused DVE STT)
  half  = exp(dw + ln(anchor/2))        (single ACT op; bias folds the mul,
                                         ACT only ever runs Exp -> no
                                         activation-table reloads)
  x1/x2 = pre -/+ half                  (DVE)
  mask  = iou > thresh                  (gpsimd, off the DVE critical path)
  kind  = tournament argmax over the 10 class planes (DVE)
"""

import numpy as np

import concourse.bass as bass
import concourse.bacc as bacc
import concourse.mybir as mybir
from concourse.tile import TileContext
from concourse.bass_utils import run_bass_kernel_spmd

F32 = mybir.dt.float32
U8 = mybir.dt.uint8
ALU = mybir.AluOpType
ACTF = mybir.ActivationFunctionType

NCORES = 8
B = 128
BL = B // NCORES  # 16

# name, H(=W), stride, G (cell-space split to widen partitions), n free chunks
HEADS = [
    ("13", 13, 32, 1, 1),
    ("26", 26, 16, 2, 1),
    ("52", 52, 8, 2, 2),
]


def _build_nc(reps=1, variant="full"):
    # Bacc (not raw Bass): its compile() legalizes multi-wait sync_info into
    # event semaphores (hardware allows one wait per instruction).
    # reps>1 wraps the body in a For_i loop -- used only for benchmarking
    # (fixed host/proxy overhead cancels between reps=1 and reps=R runs).
    # variant: "full" | "dma" (no compute) | "compute" (no chunk DMAs).
    import contextlib
    nc = bacc.Bacc("TRN2", target_bir_lowering=False)
    ins = {}
    outs = {}
    for name, H, S, G, NCH in HEADS:
        HW = H * H
        FC = HW // (G * NCH)
        ins[name] = nc.declare_dram_parameter(
            f"in{name}", [G, BL, 3, NCH, 15, FC], F32, isOutput=False)
        outs[name] = nc.declare_dram_parameter(
            f"out{name}", [G, BL, 3, NCH, 7, FC], F32, isOutput=True)
    consts = nc.declare_dram_parameter("consts", [96, 8], F32, isOutput=False)

    with TileContext(nc) as tc:
        with (
            tc.tile_pool(name="persist", bufs=1) as ppool,
            tc.tile_pool(name="work", bufs=2) as wpool,
            tc.tile_pool(name="scratch", bufs=1) as spool,
        ):
            consts_t = ppool.tile([96, 8], F32, tag="consts")
            nc.sync.dma_start(out=consts_t[:, :], in_=consts[:])

            # Per-head grid tiles (stride-scaled cell coordinates), built once
            # on gpsimd via iota.  Values <= 408, exact in f32.
            grids = {}
            for name, H, S, G, NCH in HEADS:
                P = 48 * G
                H2 = H // G
                F = H2 * H
                gx = ppool.tile([P, F], F32, tag=f"gx{name}")
                gy = ppool.tile([P, F], F32, tag=f"gy{name}")
                nc.gpsimd.iota(
                    gx[:, :], pattern=[[0, H2], [S, H]], base=0,
                    channel_multiplier=0, allow_small_or_imprecise_dtypes=True,
                )
                # iota can't write at a partition offset (and partition windows
                # must be 32-aligned), so write the g=0 pattern everywhere and
                # add the g=1 half-offset (H2*S = 208 for both split heads)
                # from consts col 7 as a one-time per-partition scalar.
                nc.gpsimd.iota(
                    gy[:, :], pattern=[[S, H2], [0, H]], base=0,
                    channel_multiplier=0, allow_small_or_imprecise_dtypes=True,
                )
                if G == 2:
                    nc.vector.tensor_scalar(
                        out=gy[:, :], in0=gy[:, :],
                        scalar1=consts_t[0:P, 7:8], scalar2=None, op0=ALU.add,
                    )
                grids[name] = (gx, gy)

            rep_ctx = tc.For_i(0, reps, 1) if reps > 1 else contextlib.nullcontext()
            with rep_ctx:
                _emit_body(nc, tc, wpool, spool, consts_t, grids, ins, outs, variant)
    nc.compile()
    return nc


def _emit_body(nc, tc, wpool, spool, consts_t, grids, ins, outs, variant="full"):
    for hi, (name, H, S, G, NCH) in enumerate(HEADS):
        HW = H * H
        P = 48 * G
        F = HW // G
        FC = F // NCH
        gx, gy = grids[name]
        in_t = ins[name]
        out_t = outs[name]
        cw, ch = 2 * hi, 2 * hi + 1

        for c in range(NCH):
            cs = c * FC
            ce = cs + FC
            it = wpool.tile([P, 15 * FC], F32, tag="in")
            ot = wpool.tile([P, 7 * FC], F32, tag="out")
            if variant != "compute":
                # one ~(15*FC*4)B descriptor per partition row
                nc.sync.dma_start(out=it[:, :], in_=in_t[:, :, :, c, :, :])

            def sl(tile, r):
                return tile[:, r * FC:(r + 1) * FC]

            if variant == "dma":
                nc.gpsimd.tensor_copy(ot[:, 0:7 * FC], it[:, 0:7 * FC])
                nc.scalar.dma_start(out=out_t[:, :, :, c, :, :], in_=ot[:, :])
                continue

            hw2 = wpool.tile([P, FC], F32, tag="hw2")
            hh2 = wpool.tile([P, FC], F32, tag="hh2")
            # ACT runs only Exp (single function table, no reloads);
            # bias folds the anchor/2 multiplier: a/2*e^x = e^(x+ln(a/2))
            nc.scalar.activation(hw2[:, :], sl(it, 3), ACTF.Exp, bias=consts_t[0:P, cw:cw + 1])
            nc.scalar.activation(hh2[:, :], sl(it, 4), ACTF.Exp, bias=consts_t[0:P, ch:ch + 1])

            # pre = grid + S*d, fused STT, in place on the dx/dy role slices
            nc.vector.scalar_tensor_tensor(
                sl(it, 1), sl(it, 1), float(S), gx[:, cs:ce], ALU.mult, ALU.add)
            nc.vector.scalar_tensor_tensor(
                sl(it, 2), sl(it, 2), float(S), gy[:, cs:ce], ALU.mult, ALU.add)
            nc.vector.tensor_tensor(sl(ot, 1), sl(it, 1), hw2[:, :], ALU.subtract)
            nc.vector.tensor_tensor(sl(ot, 3), sl(it, 1), hw2[:, :], ALU.add)
            nc.vector.tensor_tensor(sl(ot, 2), sl(it, 2), hh2[:, :], ALU.subtract)
            nc.vector.tensor_tensor(sl(ot, 4), sl(it, 2), hh2[:, :], ALU.add)

            # iou passthrough + mask (gpsimd, off the DVE critical path)
            nc.gpsimd.tensor_copy(sl(ot, 0), sl(it, 0))
            nc.gpsimd.tensor_scalar(
                out=sl(ot, 6), in0=sl(it, 0),
                scalar1=consts_t[0:P, 6:7], scalar2=None, op0=ALU.is_gt)

            # tournament argmax over class role slices 5..14.  Pool (gpsimd)
            # has no tensor_tensor support in walrus codegen, so it can only
            # take the tensor_scalar idx adds; DVE keeps cmps/maxes/selects.
            cls = [sl(it, 5 + j) for j in range(10)]
            m = []   # running max (in place on even class slices)
            ix = []  # running argmax
            for i in range(5):
                # f32 gt: Pool ts-add can't mix dtypes (and only merges' g2
                # feed copy_predicated, which needs an integer mask)
                gt = spool.tile([P, FC], F32, tag=f"gt{i}")
                idx = spool.tile([P, FC], F32, tag=f"idx{i}")
                nc.vector.tensor_tensor(gt[:, :], cls[2 * i + 1], cls[2 * i], ALU.is_gt)
                nc.vector.tensor_tensor(cls[2 * i], cls[2 * i], cls[2 * i + 1], ALU.max)
                nc.vector.tensor_scalar(
                    out=idx[:, :], in0=gt[:, :],
                    scalar1=float(2 * i), scalar2=None, op0=ALU.add)
                m.append(cls[2 * i])
                ix.append(idx[:, :])

            def merge(i, j):
                g2 = spool.tile([P, FC], U8, tag="gm")
                nc.vector.tensor_tensor(g2[:, :], m[j], m[i], ALU.is_gt)
                nc.vector.copy_predicated(ix[i], g2[:, :], ix[j])
                nc.vector.tensor_tensor(m[i], m[i], m[j], ALU.max)

            merge(0, 1)
            merge(2, 3)
            merge(0, 2)
            # final merge writes kind straight into the out tile
            gf = spool.tile([P, FC], U8, tag="gm")
            nc.vector.tensor_tensor(gf[:, :], m[4], m[0], ALU.is_gt)
            nc.vector.tensor_copy(sl(ot, 5), ix[0])
            nc.vector.copy_predicated(sl(ot, 5), gf[:, :], ix[4])

            if variant != "compute":
                nc.scalar.dma_start(out=out_t[:, :, :, c, :, :], in_=ot[:, :])


_NC_CACHE = {}


def _get_nc(reps=1, variant="full"):
    key = (reps, variant)
    if key not in _NC_CACHE:
        _NC_CACHE[key] = _build_nc(reps, variant)
    return _NC_CACHE[key]


def _host_inputs(output13, output26, output52, anchors13, anchors26, anchors52, thresh):
    consts = np.zeros((96, 8), np.float32)
    amod = np.arange(96) % 3
    for hi, anc in enumerate((anchors13, anchors26, anchors52)):
        anc = np.asarray(anc, np.float32)
        consts[:, 2 * hi] = np.log(anc[:, 0] / 2.0)[amod]
        consts[:, 2 * hi + 1] = np.log(anc[:, 1] / 2.0)[amod]
    consts[:, 6] = np.float32(thresh)
    consts[:, 7] = np.where(np.arange(96) >= 48, 208.0, 0.0)
    full = {"13": np.asarray(output13, np.float32),
            "26": np.asarray(output26, np.float32),
            "52": np.asarray(output52, np.float32)}
    # prepack: [B,45,H,W] -> (g, b, a, c, k, f) so each partition row's chunk
    # (15 roles x FC cells) is one contiguous DMA descriptor
    packed = {}
    for name, H, S, G, NCH in HEADS:
        HW = H * H
        FC = HW // (G * NCH)
        x = full[name].reshape(B, 3, 15, G, NCH, FC)  # (b, a, k, g, c, f)
        packed[name] = x.transpose(3, 0, 1, 4, 2, 5)  # (g, b, a, c, k, f)
    in_maps = []
    for cid in range(NCORES):
        mcore = {"consts": consts}
        for name, H, S, G, NCH in HEADS:
            sl = packed[name][:, cid * BL:(cid + 1) * BL]
            mcore[f"in{name}"] = np.ascontiguousarray(sl)
        in_maps.append(mcore)
    return in_maps


def _assemble(results):
    boxes_parts = []
    mask_parts = []
    for name, H, S, G, NCH in HEADS:
        HW = H * H
        FC = HW // (G * NCH)
        # [G, BL, 3, NCH, 7, FC] per core, concat batch
        ob = np.concatenate([r[f"out{name}"] for r in results], axis=1)
        # (g, b, a, c, r, f) -> (b, g, c, f, a, r) so (g,c,f) flattens to hw
        arr = ob.transpose(1, 0, 3, 5, 2, 4).reshape(B, HW, 3, 7)
        boxes_parts.append(arr[..., :6].reshape(-1, 6))
        mask_parts.append(arr[..., 6].reshape(-1) != 0)
    boxes = np.ascontiguousarray(np.concatenate(boxes_parts, 0))
    mask = np.concatenate(mask_parts, 0)
    return boxes, mask


def _run(trace=False, **inputs):
    nc = _get_nc()
    in_maps = _host_inputs(**inputs)
    res = run_bass_kernel_spmd(nc, in_maps, list(range(NCORES)), trace=trace)
    out = _assemble(res.results)
    return out, res


def kernel(**inputs):
    out, _ = _run(trace=False, **inputs)
    return out


def kernel_traced(**inputs):
    return _run(trace=True, **inputs)


# revision 42
# speedup vs baseline: 1.9733x; 1.1445x over previous
"""YOLO-style detector decode kernel for Trainium2 (8 NeuronCores, SPMD).

Sharding: pure data parallel over the batch dim (128 -> 16 per core).

Layout: for each head (13/26/52) the host prepacks the 45 channels
(3 anchors x 15 roles) into [G, B, 3, nchunk, 15, FC] so that each SBUF
partition row (g, b, a) loads its whole chunk (15 roles x FC cells) as ONE
contiguous ~40KB DMA descriptor -- real TRN2 DMA is descriptor-rate-bound
(~38ns/descriptor), so small per-role descriptors are 6x slower than the
HBM roofline.  Outputs are likewise packed as [G, B, 3, nchunk, 7, FC]
(iou, x1, y1, x2, y2, kind, mask) and unpacked on the host with cheap numpy
transposes.

Decode per cell-anchor (partition p = g*48 + b*3 + a, free = cells):

  pre   = grid*stride + stride*d        (gpsimd iota grid + one fused DVE STT)
  half  = exp(dw + ln(anchor/2))        (single ACT op; bias folds the mul,
                                         ACT only ever runs Exp -> no
                                         activation-table reloads)
  x1/x2 = pre -/+ half                  (DVE)
  mask  = iou > thresh                  (gpsimd, off the DVE critical path)
  kind  = tournament argmax over the 10 class planes (DVE)
"""

import numpy as np

import concourse.bass as bass
import concourse.bacc as bacc
import concourse.mybir as mybir
from concourse.tile import TileContext
from concourse.bass_utils import run_bass_kernel_spmd

F32 = mybir.dt.float32
U8 = mybir.dt.uint8
ALU = mybir.AluOpType
ACTF = mybir.ActivationFunctionType

NCORES = 8
B = 128
BL = B // NCORES  # 16

# name, H(=W), stride, G (cell-space split to widen partitions), n free chunks
HEADS = [
    ("13", 13, 32, 1, 1),
    ("26", 26, 16, 2, 1),
    ("52", 52, 8, 2, 2),
]


def _build_nc(reps=1, variant="full"):
    # Bacc (not raw Bass): its compile() legalizes multi-wait sync_info into
    # event semaphores (hardware allows one wait per instruction).
    # reps>1 wraps the body in a For_i loop -- used only for benchmarking
    # (fixed host/proxy overhead cancels between reps=1 and reps=R runs).
    # variant: "full" | "dma" (no compute) | "compute" (no chunk DMAs).
    import contextlib
    nc = bacc.Bacc("TRN2", target_bir_lowering=False)
    ins = {}
    outs = {}
    for name, H, S, G, NCH in HEADS:
        HW = H * H
        FC = HW // (G * NCH)
        ins[name] = nc.declare_dram_parameter(
            f"in{name}", [G, BL, 3, NCH, 15, FC], F32, isOutput=False)
        outs[name] = nc.declare_dram_parameter(
            f"out{name}", [G, BL, 3, NCH, 7, FC], F32, isOutput=True)
    consts = nc.declare_dram_parameter("consts", [96, 8], F32, isOutput=False)

    with TileContext(nc) as tc:
        with (
            tc.tile_pool(name="persist", bufs=1) as ppool,
            tc.tile_pool(name="work", bufs=2) as wpool,
            tc.tile_pool(name="scratch", bufs=1) as spool,
        ):
            consts_t = ppool.tile([96, 8], F32, tag="consts")
            nc.sync.dma_start(out=consts_t[:, :], in_=consts[:])

            # Per-head grid tiles (stride-scaled cell coordinates), built once
            # on gpsimd via iota.  Values <= 408, exact in f32.
            grids = {}
            for name, H, S, G, NCH in HEADS:
                P = 48 * G
                H2 = H // G
                F = H2 * H
                gx = ppool.tile([P, F], F32, tag=f"gx{name}")
                gy = ppool.tile([P, F], F32, tag=f"gy{name}")
                nc.gpsimd.iota(
                    gx[:, :], pattern=[[0, H2], [S, H]], base=0,
                    channel_multiplier=0, allow_small_or_imprecise_dtypes=True,
                )
                # iota can't write at a partition offset (and partition windows
                # must be 32-aligned), so write the g=0 pattern everywhere and
                # add the g=1 half-offset (H2*S = 208 for both split heads)
                # from consts col 7 as a one-time per-partition scalar.
                nc.gpsimd.iota(
                    gy[:, :], pattern=[[S, H2], [0, H]], base=0,
                    channel_multiplier=0, allow_small_or_imprecise_dtypes=True,
                )
                if G == 2:
                    nc.vector.tensor_scalar(
                        out=gy[:, :], in0=gy[:, :],
                        scalar1=consts_t[0:P, 7:8], scalar2=None, op0=ALU.add,
                    )
                grids[name] = (gx, gy)

            rep_ctx = tc.For_i(0, reps, 1) if reps > 1 else contextlib.nullcontext()
            with rep_ctx:
                _emit_body(nc, tc, wpool, spool, consts_t, grids, ins, outs, variant)
    nc.compile()
    return nc


def _emit_body(nc, tc, wpool, spool, consts_t, grids, ins, outs, variant="full"):
    for hi, (name, H, S, G, NCH) in enumerate(HEADS):
        HW = H * H
        P = 48 * G
        F = HW // G
        FC = F // NCH
        gx, gy = grids[name]
        in_t = ins[name]
        out_t = outs[name]
        cw, ch = 2 * hi, 2 * hi + 1

        for c in range(NCH):
            cs = c * FC
            ce = cs + FC
            it = wpool.tile([P, 15 * FC], F32, tag="in")
            ot = wpool.tile([P, 7 * FC], F32, tag="out")
            if variant != "compute":
                # one ~(15*FC*4)B descriptor per partition row; split the
                # load across both HWDGE queues to balance bytes (the store
                # path on scalar carries ~half the input volume)
                nc.sync.dma_start(out=it[:, 0:4 * FC], in_=in_t[:, :, :, c, 0:4, :])
                nc.scalar.dma_start(out=it[:, 4 * FC:], in_=in_t[:, :, :, c, 4:, :])

            def sl(tile, r):
                return tile[:, r * FC:(r + 1) * FC]

            if variant == "dma":
                nc.gpsimd.tensor_copy(ot[:, 0:7 * FC], it[:, 0:7 * FC])
                nc.scalar.dma_start(out=out_t[:, :, :, c, :, :], in_=ot[:, :])
                continue

            hw2 = wpool.tile([P, FC], F32, tag="hw2")
            hh2 = wpool.tile([P, FC], F32, tag="hh2")
            # ACT runs only Exp (single function table, no reloads);
            # bias folds the anchor/2 multiplier: a/2*e^x = e^(x+ln(a/2))
            nc.scalar.activation(hw2[:, :], sl(it, 3), ACTF.Exp, bias=consts_t[0:P, cw:cw + 1])
            nc.scalar.activation(hh2[:, :], sl(it, 4), ACTF.Exp, bias=consts_t[0:P, ch:ch + 1])

            # pre = grid + S*d, fused STT, in place on the dx/dy role slices
            nc.vector.scalar_tensor_tensor(
                sl(it, 1), sl(it, 1), float(S), gx[:, cs:ce], ALU.mult, ALU.add)
            nc.vector.scalar_tensor_tensor(
                sl(it, 2), sl(it, 2), float(S), gy[:, cs:ce], ALU.mult, ALU.add)
            nc.vector.tensor_tensor(sl(ot, 1), sl(it, 1), hw2[:, :], ALU.subtract)
            nc.vector.tensor_tensor(sl(ot, 3), sl(it, 1), hw2[:, :], ALU.add)
            nc.vector.tensor_tensor(sl(ot, 2), sl(it, 2), hh2[:, :], ALU.subtract)
            nc.vector.tensor_tensor(sl(ot, 4), sl(it, 2), hh2[:, :], ALU.add)

            # iou passthrough + mask (gpsimd, off the DVE critical path)
            nc.gpsimd.tensor_copy(sl(ot, 0), sl(it, 0))
            nc.gpsimd.tensor_scalar(
                out=sl(ot, 6), in0=sl(it, 0),
                scalar1=consts_t[0:P, 6:7], scalar2=None, op0=ALU.is_gt)

            # tournament argmax over class role slices 5..14.  Pool (gpsimd)
            # has no tensor_tensor support in walrus codegen, so it can only
            # take the tensor_scalar idx adds; DVE keeps cmps/maxes/selects.
            cls = [sl(it, 5 + j) for j in range(10)]
            m = []   # running max (in place on even class slices)
            ix = []  # running argmax
            for i in range(5):
                # f32 gt: Pool ts-add can't mix dtypes (and only merges' g2
                # feed copy_predicated, which needs an integer mask)
                gt = spool.tile([P, FC], F32, tag=f"gt{i}")
                idx = spool.tile([P, FC], F32, tag=f"idx{i}")
                nc.vector.tensor_tensor(gt[:, :], cls[2 * i + 1], cls[2 * i], ALU.is_gt)
                nc.vector.tensor_tensor(cls[2 * i], cls[2 * i], cls[2 * i + 1], ALU.max)
                nc.vector.tensor_scalar(
                    out=idx[:, :], in0=gt[:, :],
                    scalar1=float(2 * i), scalar2=None, op0=ALU.add)
                m.append(cls[2 * i])
                ix.append(idx[:, :])

            def merge(i, j):
                g2 = spool.tile([P, FC], U8, tag="gm")
                nc.vector.tensor_tensor(g2[:, :], m[j], m[i], ALU.is_gt)
                nc.vector.copy_predicated(ix[i], g2[:, :], ix[j])
                nc.vector.tensor_tensor(m[i], m[i], m[j], ALU.max)

            merge(0, 1)
            merge(2, 3)
            merge(0, 2)
            # final merge writes kind straight into the out tile
            gf = spool.tile([P, FC], U8, tag="gm")
            nc.vector.tensor_tensor(gf[:, :], m[4], m[0], ALU.is_gt)
            nc.vector.tensor_copy(sl(ot, 5), ix[0])
            nc.vector.copy_predicated(sl(ot, 5), gf[:, :], ix[4])

            if variant != "compute":
                nc.sync.dma_start(out=out_t[:, :, :, c, :, :], in_=ot[:, :])


_NC_CACHE = {}


def _get_nc(reps=1, variant="full"):
    key = (reps, variant)
    if key not in _NC_CACHE:
        _NC_CACHE[key] = _build_nc(reps, variant)
    return _NC_CACHE[key]


def _host_inputs(output13, output26, output52, anchors13, anchors26, anchors52, thresh):
    consts = np.zeros((96, 8), np.float32)
    amod = np.arange(96) % 3
    for hi, anc in enumerate((anchors13, anchors26, anchors52)):
        anc = np.asarray(anc, np.float32)
        consts[:, 2 * hi] = np.log(anc[:, 0] / 2.0)[amod]
        consts[:, 2 * hi + 1] = np.log(anc[:, 1] / 2.0)[amod]
    consts[:, 6] = np.float32(thresh)
    consts[:, 7] = np.where(np.arange(96) >= 48, 208.0, 0.0)
    full = {"13": np.asarray(output13, np.float32),
            "26": np.asarray(output26, np.float32),
            "52": np.asarray(output52, np.float32)}
    # prepack: [B,45,H,W] -> (g, b, a, c, k, f) so each partition row's chunk
    # (15 roles x FC cells) is one contiguous DMA descriptor
    packed = {}
    for name, H, S, G, NCH in HEADS:
        HW = H * H
        FC = HW // (G * NCH)
        x = full[name].reshape(B, 3, 15, G, NCH, FC)  # (b, a, k, g, c, f)
        packed[name] = x.transpose(3, 0, 1, 4, 2, 5)  # (g, b, a, c, k, f)
    in_maps = []
    for cid in range(NCORES):
        mcore = {"consts": consts}
        for name, H, S, G, NCH in HEADS:
            sl = packed[name][:, cid * BL:(cid + 1) * BL]
            mcore[f"in{name}"] = np.ascontiguousarray(sl)
        in_maps.append(mcore)
    return in_maps


def _assemble(results):
    boxes_parts = []
    mask_parts = []
    for name, H, S, G, NCH in HEADS:
        HW = H * H
        FC = HW // (G * NCH)
        # [G, BL, 3, NCH, 7, FC] per core, concat batch
        ob = np.concatenate([r[f"out{name}"] for r in results], axis=1)
        # (g, b, a, c, r, f) -> (b, g, c, f, a, r) so (g,c,f) flattens to hw
        arr = ob.transpose(1, 0, 3, 5, 2, 4).reshape(B, HW, 3, 7)
        boxes_parts.append(arr[..., :6].reshape(-1, 6))
        mask_parts.append(arr[..., 6].reshape(-1) != 0)
    boxes = np.ascontiguousarray(np.concatenate(boxes_parts, 0))
    mask = np.concatenate(mask_parts, 0)
    return boxes, mask


def _run(trace=False, **inputs):
    nc = _get_nc()
    in_maps = _host_inputs(**inputs)
    res = run_bass_kernel_spmd(nc, in_maps, list(range(NCORES)), trace=trace)
    out = _assemble(res.results)
    return out, res


def kernel(**inputs):
    out, _ = _run(trace=False, **inputs)
    return out


def kernel_traced(**inputs):
    return _run(trace=True, **inputs)


# revision 48
# speedup vs baseline: 2.1851x; 1.1073x over previous
"""YOLO-style detector decode kernel for Trainium2 (8 NeuronCores, SPMD).

Sharding: pure data parallel over the batch dim (128 -> 16 per core).

Layout: for each head (13/26/52) the host prepacks the 45 channels
(3 anchors x 15 roles) into [G, B, 3, nchunk, 15, FC] so that each SBUF
partition row (g, b, a) loads its whole chunk (15 roles x FC cells) as ONE
contiguous ~40KB DMA descriptor -- real TRN2 DMA is descriptor-rate-bound
(~38ns/descriptor), so small per-role descriptors are 6x slower than the
HBM roofline.  Outputs are likewise packed as [G, B, 3, nchunk, 7, FC]
(iou, x1, y1, x2, y2, kind, mask) and unpacked on the host with cheap numpy
transposes.

Decode per cell-anchor (partition p = g*48 + b*3 + a, free = cells):

  pre   = grid*stride + stride*d        (gpsimd iota grid + one fused DVE STT)
  half  = exp(dw + ln(anchor/2))        (single ACT op; bias folds the mul,
                                         ACT only ever runs Exp -> no
                                         activation-table reloads)
  x1/x2 = pre -/+ half                  (DVE)
  mask  = iou > thresh                  (gpsimd, off the DVE critical path)
  kind  = tournament argmax over the 10 class planes (DVE)
"""

import numpy as np

import concourse.bass as bass
import concourse.bacc as bacc
import concourse.mybir as mybir
from concourse.tile import TileContext
from concourse.bass_utils import run_bass_kernel_spmd

F32 = mybir.dt.float32
U8 = mybir.dt.uint8
ALU = mybir.AluOpType
ACTF = mybir.ActivationFunctionType

NCORES = 8
B = 128
BL = B // NCORES  # 16

# name, H(=W), stride, G (cell-space split to widen partitions), n free chunks
HEADS = [
    ("13", 13, 32, 1, 1),
    ("26", 26, 16, 2, 1),
    ("52", 52, 8, 2, 2),
]


def _build_nc(reps=1, variant="full"):
    # Bacc (not raw Bass): its compile() legalizes multi-wait sync_info into
    # event semaphores (hardware allows one wait per instruction).
    # reps>1 wraps the body in a For_i loop -- used only for benchmarking
    # (fixed host/proxy overhead cancels between reps=1 and reps=R runs).
    # variant: "full" | "dma" (no compute) | "compute" (no chunk DMAs).
    import contextlib
    nc = bacc.Bacc("TRN2", target_bir_lowering=False)
    ins = {}
    outs = {}
    for name, H, S, G, NCH in HEADS:
        HW = H * H
        FC = HW // (G * NCH)
        ins[name] = nc.declare_dram_parameter(
            f"in{name}", [G, BL, 3, NCH, 15, FC], F32, isOutput=False)
        outs[name] = nc.declare_dram_parameter(
            f"out{name}", [G, BL, 3, NCH, 7, FC], F32, isOutput=True)
    consts = nc.declare_dram_parameter("consts", [96, 8], F32, isOutput=False)

    with TileContext(nc) as tc:
        with (
            tc.tile_pool(name="persist", bufs=1) as ppool,
            tc.tile_pool(name="work", bufs=2) as wpool,
            tc.tile_pool(name="scratch", bufs=1) as spool,
        ):
            consts_t = ppool.tile([96, 8], F32, tag="consts")
            nc.sync.dma_start(out=consts_t[:, :], in_=consts[:])

            # Per-head grid tiles (stride-scaled cell coordinates), built once
            # on gpsimd via iota.  Values <= 408, exact in f32.
            grids = {}
            for name, H, S, G, NCH in HEADS:
                P = 48 * G
                H2 = H // G
                F = H2 * H
                gx = ppool.tile([P, F], F32, tag=f"gx{name}")
                gy = ppool.tile([P, F], F32, tag=f"gy{name}")
                nc.gpsimd.iota(
                    gx[:, :], pattern=[[0, H2], [S, H]], base=0,
                    channel_multiplier=0, allow_small_or_imprecise_dtypes=True,
                )
                # iota can't write at a partition offset (and partition windows
                # must be 32-aligned), so write the g=0 pattern everywhere and
                # add the g=1 half-offset (H2*S = 208 for both split heads)
                # from consts col 7 as a one-time per-partition scalar.
                nc.gpsimd.iota(
                    gy[:, :], pattern=[[S, H2], [0, H]], base=0,
                    channel_multiplier=0, allow_small_or_imprecise_dtypes=True,
                )
                if G == 2:
                    nc.vector.tensor_scalar(
                        out=gy[:, :], in0=gy[:, :],
                        scalar1=consts_t[0:P, 7:8], scalar2=None, op0=ALU.add,
                    )
                grids[name] = (gx, gy)

            rep_ctx = (
                tc.For_i(0, reps, 1, hint_engines=(mybir.EngineType.DVE,
                                                   mybir.EngineType.SP,
                                                   mybir.EngineType.Activation))
                if reps > 1 else contextlib.nullcontext()
            )
            with rep_ctx:
                _emit_body(nc, tc, wpool, spool, consts_t, grids, ins, outs, variant)
    nc.compile()
    return nc


def _emit_body(nc, tc, wpool, spool, consts_t, grids, ins, outs, variant="full"):
    for hi, (name, H, S, G, NCH) in enumerate(HEADS):
        HW = H * H
        P = 48 * G
        F = HW // G
        FC = F // NCH
        gx, gy = grids[name]
        in_t = ins[name]
        out_t = outs[name]
        cw, ch = 2 * hi, 2 * hi + 1

        for c in range(NCH):
            cs = c * FC
            ce = cs + FC
            it = wpool.tile([P, 15 * FC], F32, tag="in")
            # packed planes: x1, y1, x2, y2, kind, mask (iou is DMA'd from it)
            ot = wpool.tile([P, 6 * FC], F32, tag="out")
            if variant != "compute":
                # one ~(15*FC*4)B descriptor per partition row; split the
                # load across both HWDGE queues to balance bytes (the store
                # path on scalar carries ~half the input volume)
                nc.sync.dma_start(out=it[:, 0:4 * FC], in_=in_t[:, :, :, c, 0:4, :])
                nc.scalar.dma_start(out=it[:, 4 * FC:], in_=in_t[:, :, :, c, 4:, :])

            def sl(tile, r):
                return tile[:, r * FC:(r + 1) * FC]

            if variant == "dma":
                nc.sync.dma_start(out=out_t[:, :, :, c, 0:6, :], in_=it[:, 0:6 * FC])
                nc.sync.dma_start(out=out_t[:, :, :, c, 6:7, :], in_=it[:, 0:FC])
                continue

            hw2 = wpool.tile([P, FC], F32, tag="hw2")
            hh2 = wpool.tile([P, FC], F32, tag="hh2")
            # ACT runs only Exp (single function table, no reloads);
            # bias folds the anchor/2 multiplier: a/2*e^x = e^(x+ln(a/2))
            nc.scalar.activation(hw2[:, :], sl(it, 3), ACTF.Exp, bias=consts_t[0:P, cw:cw + 1])
            nc.scalar.activation(hh2[:, :], sl(it, 4), ACTF.Exp, bias=consts_t[0:P, ch:ch + 1])

            # pre = grid + S*d, fused STT, in place on the dx/dy role slices
            nc.vector.scalar_tensor_tensor(
                sl(it, 1), sl(it, 1), float(S), gx[:, cs:ce], ALU.mult, ALU.add)
            nc.vector.scalar_tensor_tensor(
                sl(it, 2), sl(it, 2), float(S), gy[:, cs:ce], ALU.mult, ALU.add)
            nc.vector.tensor_tensor(sl(ot, 0), sl(it, 1), hw2[:, :], ALU.subtract)
            nc.vector.tensor_tensor(sl(ot, 2), sl(it, 1), hw2[:, :], ALU.add)
            nc.vector.tensor_tensor(sl(ot, 1), sl(it, 2), hh2[:, :], ALU.subtract)
            nc.vector.tensor_tensor(sl(ot, 3), sl(it, 2), hh2[:, :], ALU.add)

            # mask (gpsimd, off the DVE critical path)
            nc.gpsimd.tensor_scalar(
                out=sl(ot, 5), in0=sl(it, 0),
                scalar1=consts_t[0:P, 6:7], scalar2=None, op0=ALU.is_gt)

            # tournament argmax over class role slices 5..14.  Pool (gpsimd)
            # has no tensor_tensor support in walrus codegen, so it can only
            # take the tensor_scalar idx adds; DVE keeps cmps/maxes/selects.
            cls = [sl(it, 5 + j) for j in range(10)]
            m = []   # running max (in place on even class slices)
            ix = []  # running argmax
            for i in range(5):
                # f32 gt: Pool ts-add can't mix dtypes (and only merges' g2
                # feed copy_predicated, which needs an integer mask)
                gt = spool.tile([P, FC], F32, tag=f"gt{i}")
                idx = spool.tile([P, FC], F32, tag=f"idx{i}")
                nc.vector.tensor_tensor(gt[:, :], cls[2 * i + 1], cls[2 * i], ALU.is_gt)
                nc.vector.tensor_tensor(cls[2 * i], cls[2 * i], cls[2 * i + 1], ALU.max)
                nc.vector.tensor_scalar(
                    out=idx[:, :], in0=gt[:, :],
                    scalar1=float(2 * i), scalar2=None, op0=ALU.add)
                m.append(cls[2 * i])
                ix.append(idx[:, :])

            def merge(i, j):
                g2 = spool.tile([P, FC], U8, tag="gm")
                nc.vector.tensor_tensor(g2[:, :], m[j], m[i], ALU.is_gt)
                nc.vector.copy_predicated(ix[i], g2[:, :], ix[j])
                nc.vector.tensor_tensor(m[i], m[i], m[j], ALU.max)

            merge(0, 1)
            merge(2, 3)
            merge(0, 2)
            # final merge writes kind straight into the out tile
            gf = spool.tile([P, FC], U8, tag="gm")
            nc.vector.tensor_tensor(gf[:, :], m[4], m[0], ALU.is_gt)
            nc.vector.tensor_copy(sl(ot, 4), ix[0])
            nc.vector.copy_predicated(sl(ot, 4), gf[:, :], ix[4])

            if variant != "compute":
                nc.sync.dma_start(out=out_t[:, :, :, c, 0:6, :], in_=ot[:, :])
                nc.sync.dma_start(out=out_t[:, :, :, c, 6:7, :], in_=sl(it, 0))


_NC_CACHE = {}


def _get_nc(reps=1, variant="full"):
    key = (reps, variant)
    if key not in _NC_CACHE:
        _NC_CACHE[key] = _build_nc(reps, variant)
    return _NC_CACHE[key]


def _host_inputs(output13, output26, output52, anchors13, anchors26, anchors52, thresh):
    consts = np.zeros((96, 8), np.float32)
    amod = np.arange(96) % 3
    for hi, anc in enumerate((anchors13, anchors26, anchors52)):
        anc = np.asarray(anc, np.float32)
        consts[:, 2 * hi] = np.log(anc[:, 0] / 2.0)[amod]
        consts[:, 2 * hi + 1] = np.log(anc[:, 1] / 2.0)[amod]
    consts[:, 6] = np.float32(thresh)
    consts[:, 7] = np.where(np.arange(96) >= 48, 208.0, 0.0)
    full = {"13": np.asarray(output13, np.float32),
            "26": np.asarray(output26, np.float32),
            "52": np.asarray(output52, np.float32)}
    # prepack: [B,45,H,W] -> (g, b, a, c, k, f) so each partition row's chunk
    # (15 roles x FC cells) is one contiguous DMA descriptor
    packed = {}
    for name, H, S, G, NCH in HEADS:
        HW = H * H
        FC = HW // (G * NCH)
        x = full[name].reshape(B, 3, 15, G, NCH, FC)  # (b, a, k, g, c, f)
        packed[name] = x.transpose(3, 0, 1, 4, 2, 5)  # (g, b, a, c, k, f)
    in_maps = []
    for cid in range(NCORES):
        mcore = {"consts": consts}
        for name, H, S, G, NCH in HEADS:
            sl = packed[name][:, cid * BL:(cid + 1) * BL]
            mcore[f"in{name}"] = np.ascontiguousarray(sl)
        in_maps.append(mcore)
    return in_maps


def _assemble(results):
    boxes_parts = []
    mask_parts = []
    for name, H, S, G, NCH in HEADS:
        HW = H * H
        FC = HW // (G * NCH)
        # [G, BL, 3, NCH, 7, FC] per core, concat batch
        ob = np.concatenate([r[f"out{name}"] for r in results], axis=1)
        # (g, b, a, c, r, f) -> (b, g, c, f, a, r) so (g,c,f) flattens to hw
        arr = ob.transpose(1, 0, 3, 5, 2, 4).reshape(B, HW, 3, 7)
        # packed plane order: x1, y1, x2, y2, kind, mask, iou
        boxes_parts.append(arr[..., [6, 0, 1, 2, 3, 4]].reshape(-1, 6))
        mask_parts.append(arr[..., 5].reshape(-1) != 0)
    boxes = np.ascontiguousarray(np.concatenate(boxes_parts, 0))
    mask = np.concatenate(mask_parts, 0)
    return boxes, mask


def _run(trace=False, **inputs):
    nc = _get_nc()
    in_maps = _host_inputs(**inputs)
    res = run_bass_kernel_spmd(nc, in_maps, list(range(NCORES)), trace=trace)
    out = _assemble(res.results)
    return out, res


def kernel(**inputs):
    out, _ = _run(trace=False, **inputs)
    return out


def kernel_traced(**inputs):
    return _run(trace=True, **inputs)


# revision 58
# speedup vs baseline: 3.6246x; 1.6588x over previous
"""YOLO-style detector decode kernel for Trainium2 (8 NeuronCores, SPMD).

Sharding: pure data parallel over the batch dim (128 -> 16 per core).

Layout: for each head (13/26/52) the host prepacks the 45 channels
(3 anchors x 15 roles) into [G, B, 3, nchunk, 15, FC] so that each SBUF
partition row (g, b, a) loads its whole chunk (15 roles x FC cells) as ONE
contiguous ~40KB DMA descriptor -- real TRN2 DMA is descriptor-rate-bound
(~38ns/descriptor), so small per-role descriptors are 6x slower than the
HBM roofline.  Outputs are likewise packed as [G, B, 3, nchunk, 7, FC]
(iou, x1, y1, x2, y2, kind, mask) and unpacked on the host with cheap numpy
transposes.

Decode per cell-anchor (partition p = g*48 + b*3 + a, free = cells):

  pre   = grid*stride + stride*d        (gpsimd iota grid + one fused DVE STT)
  half  = exp(dw + ln(anchor/2))        (single ACT op; bias folds the mul,
                                         ACT only ever runs Exp -> no
                                         activation-table reloads)
  x1/x2 = pre -/+ half                  (DVE)
  mask  = iou > thresh                  (gpsimd, off the DVE critical path)
  kind  = tournament argmax over the 10 class planes (DVE)
"""

import numpy as np

import concourse.bass as bass
import concourse.bacc as bacc
import concourse.mybir as mybir
from concourse.bass import MemorySpace
from concourse.tile import TileContext
from concourse.bass_utils import run_bass_kernel_spmd

F32 = mybir.dt.float32
U8 = mybir.dt.uint8
ALU = mybir.AluOpType
ACTF = mybir.ActivationFunctionType

NCORES = 8
B = 128
BL = B // NCORES  # 16

# name, H(=W), stride, G (cell-space split to widen partitions), n free chunks
HEADS = [
    ("13", 13, 32, 1, 1),
    ("26", 26, 16, 2, 1),
    ("52", 52, 8, 2, 2),
]


def _build_nc(reps=1, variant="full"):
    # Bacc (not raw Bass): its compile() legalizes multi-wait sync_info into
    # event semaphores (hardware allows one wait per instruction).
    # reps>1 wraps the body in a For_i loop -- used only for benchmarking
    # (fixed host/proxy overhead cancels between reps=1 and reps=R runs).
    # variant: "full" | "dma" (no compute) | "compute" (no chunk DMAs).
    import contextlib
    nc = bacc.Bacc("TRN2", target_bir_lowering=False)
    ins = {}
    outs = {}
    for name, H, S, G, NCH in HEADS:
        HW = H * H
        FC = HW // (G * NCH)
        ins[name] = nc.declare_dram_parameter(
            f"in{name}", [G, BL, 3, NCH, 15, FC], F32, isOutput=False)
        outs[name] = nc.declare_dram_parameter(
            f"out{name}", [G, BL, 3, NCH, 7, FC], F32, isOutput=True)
    consts = nc.declare_dram_parameter("consts", [96, 8], F32, isOutput=False)
    # identity weight matrices for PE linear combines: I, -I, 32I, 16I, 8I
    idents = nc.declare_dram_parameter("idents", [5, 96, 96], F32, isOutput=False)

    with TileContext(nc) as tc:
        with (
            tc.tile_pool(name="persist", bufs=1) as ppool,
            tc.tile_pool(name="work", bufs=2) as wpool,
            tc.tile_pool(name="scratch", bufs=1) as spool,
            tc.tile_pool(name="psum", bufs=1, space=MemorySpace.PSUM) as qpool,
        ):
            consts_t = ppool.tile([96, 8], F32, tag="consts")
            nc.sync.dma_start(out=consts_t[:, :], in_=consts[:])
            id_t = ppool.tile([96, 5 * 96], F32, tag="idents")
            nc.sync.dma_start(out=id_t[:, :], in_=idents[:].rearrange("v r c -> r v c"))
            id_sl = {"pos": id_t[:, 0:96], "neg": id_t[:, 96:192],
                     32: id_t[:, 192:288], 16: id_t[:, 288:384], 8: id_t[:, 384:480]}

            # Per-head grid tiles (stride-scaled cell coordinates), built once
            # on gpsimd via iota.  Values <= 408, exact in f32.
            grids = {}
            for name, H, S, G, NCH in HEADS:
                P = 48 * G
                H2 = H // G
                F = H2 * H
                gx = ppool.tile([P, F], F32, tag=f"gx{name}")
                gy = ppool.tile([P, F], F32, tag=f"gy{name}")
                nc.gpsimd.iota(
                    gx[:, :], pattern=[[0, H2], [S, H]], base=0,
                    channel_multiplier=0, allow_small_or_imprecise_dtypes=True,
                )
                # iota can't write at a partition offset (and partition windows
                # must be 32-aligned), so write the g=0 pattern everywhere and
                # add the g=1 half-offset (H2*S = 208 for both split heads)
                # from consts col 7 as a one-time per-partition scalar.
                nc.gpsimd.iota(
                    gy[:, :], pattern=[[S, H2], [0, H]], base=0,
                    channel_multiplier=0, allow_small_or_imprecise_dtypes=True,
                )
                if G == 2:
                    nc.vector.tensor_scalar(
                        out=gy[:, :], in0=gy[:, :],
                        scalar1=consts_t[0:P, 7:8], scalar2=None, op0=ALU.add,
                    )
                grids[name] = (gx, gy)

            rep_ctx = (
                tc.For_i(0, reps, 1, hint_engines=(mybir.EngineType.DVE,
                                                   mybir.EngineType.SP,
                                                   mybir.EngineType.Activation))
                if reps > 1 else contextlib.nullcontext()
            )
            with rep_ctx:
                _emit_body(nc, tc, wpool, spool, qpool, id_sl, consts_t, grids,
                           ins, outs, variant)
    nc.compile()
    return nc


def _emit_body(nc, tc, wpool, spool, qpool, id_sl, consts_t, grids, ins, outs,
               variant="full"):
    for hi, (name, H, S, G, NCH) in enumerate(HEADS):
        HW = H * H
        P = 48 * G
        F = HW // G
        FC = F // NCH
        gx, gy = grids[name]
        in_t = ins[name]
        out_t = outs[name]
        cw, ch = 2 * hi, 2 * hi + 1

        for c in range(NCH):
            cs = c * FC
            ce = cs + FC
            it = wpool.tile([P, 15 * FC], F32, tag="in")
            # packed planes: x1, y1, x2, y2, kind, mask (iou is DMA'd from it)
            ot = wpool.tile([P, 6 * FC], F32, tag="out")
            if variant != "compute":
                # one ~(15*FC*4)B descriptor per partition row; split the
                # load across both HWDGE queues to balance bytes (the store
                # path on scalar carries ~half the input volume)
                nc.sync.dma_start(out=it[:, 0:4 * FC], in_=in_t[:, :, :, c, 0:4, :])
                nc.scalar.dma_start(out=it[:, 4 * FC:], in_=in_t[:, :, :, c, 4:, :])

            def sl(tile, r):
                return tile[:, r * FC:(r + 1) * FC]

            if variant == "dma":
                nc.sync.dma_start(out=out_t[:, :, :, c, 0:6, :], in_=it[:, 0:6 * FC])
                nc.sync.dma_start(out=out_t[:, :, :, c, 6:7, :], in_=it[:, 0:FC])
                continue

            hw2 = wpool.tile([P, FC], F32, tag="hw2")
            hh2 = wpool.tile([P, FC], F32, tag="hh2")
            # ACT runs only Exp (single function table, no reloads);
            # bias folds the anchor/2 multiplier: a/2*e^x = e^(x+ln(a/2))
            nc.scalar.activation(hw2[:, :], sl(it, 3), ACTF.Exp, bias=consts_t[0:P, cw:cw + 1])
            nc.scalar.activation(hh2[:, :], sl(it, 4), ACTF.Exp, bias=consts_t[0:P, ch:ch + 1])

            # x1/x2/y1/y2 = S*d + grid -/+ half: linear combines on the (idle)
            # TensorEngine via identity-weight matmul accumulation into PSUM,
            # then ACT copies PSUM -> packed out tile.  Halved for N<=512.
            i_pos, i_neg, i_s = id_sl["pos"], id_sl["neg"], id_sl[S]
            nh = 2 if FC > 512 else 1
            HF = FC // nh
            for h in range(nh):
                hs, he = h * HF, (h + 1) * HF
                for pi, (d_r, grid, half, sgn) in enumerate((
                        (1, gx, hw2, "neg"),   # x1 = S*dx + gx - hw2
                        (2, gy, hh2, "neg"),   # y1
                        (1, gx, hw2, "pos"),   # x2 = S*dx + gx + hw2
                        (2, gy, hh2, "pos"),   # y2
                )):
                    pt = qpool.tile([P, HF], F32, tag=f"ps{pi}{h}")
                    dsl = it[:, d_r * FC + hs:d_r * FC + he]
                    nc.tensor.matmul(pt[:, :], i_s[0:P, 0:P], dsl, start=True, stop=False)
                    nc.tensor.matmul(pt[:, :], i_pos[0:P, 0:P], grid[:, cs + hs:cs + he],
                                     start=False, stop=False)
                    nc.tensor.matmul(pt[:, :], id_sl[sgn][0:P, 0:P], half[:, hs:he],
                                     start=False, stop=True)
                    nc.scalar.activation(ot[:, pi * FC + hs:pi * FC + he], pt[:, :],
                                         ACTF.Copy)

            # mask (gpsimd, off the DVE critical path)
            nc.gpsimd.tensor_scalar(
                out=sl(ot, 5), in0=sl(it, 0),
                scalar1=consts_t[0:P, 6:7], scalar2=None, op0=ALU.is_gt)

            # tournament argmax over class role slices 5..14.  Pool (gpsimd)
            # has no tensor_tensor support in walrus codegen, so it can only
            # take the tensor_scalar idx adds; DVE keeps cmps/maxes/selects.
            cls = [sl(it, 5 + j) for j in range(10)]
            m = []   # running max (in place on even class slices)
            ix = []  # running argmax
            for i in range(5):
                # f32 gt: Pool ts-add can't mix dtypes (and only merges' g2
                # feed copy_predicated, which needs an integer mask)
                gt = spool.tile([P, FC], F32, tag=f"gt{i}")
                idx = spool.tile([P, FC], F32, tag=f"idx{i}")
                nc.vector.tensor_tensor(gt[:, :], cls[2 * i + 1], cls[2 * i], ALU.is_gt)
                nc.vector.tensor_tensor(cls[2 * i], cls[2 * i], cls[2 * i + 1], ALU.max)
                nc.vector.tensor_scalar(
                    out=idx[:, :], in0=gt[:, :],
                    scalar1=float(2 * i), scalar2=None, op0=ALU.add)
                m.append(cls[2 * i])
                ix.append(idx[:, :])

            def merge(i, j):
                g2 = spool.tile([P, FC], U8, tag="gm")
                nc.vector.tensor_tensor(g2[:, :], m[j], m[i], ALU.is_gt)
                nc.vector.copy_predicated(ix[i], g2[:, :], ix[j])
                nc.vector.tensor_tensor(m[i], m[i], m[j], ALU.max)

            merge(0, 1)
            merge(2, 3)
            merge(0, 2)
            # final merge writes kind straight into the out tile
            gf = spool.tile([P, FC], U8, tag="gm")
            nc.vector.tensor_tensor(gf[:, :], m[4], m[0], ALU.is_gt)
            nc.vector.tensor_copy(sl(ot, 4), ix[0])
            nc.vector.copy_predicated(sl(ot, 4), gf[:, :], ix[4])

            if variant != "compute":
                nc.sync.dma_start(out=out_t[:, :, :, c, 0:6, :], in_=ot[:, :])
                nc.sync.dma_start(out=out_t[:, :, :, c, 6:7, :], in_=sl(it, 0))


_NC_CACHE = {}


def _get_nc(reps=1, variant="full"):
    key = (reps, variant)
    if key not in _NC_CACHE:
        _NC_CACHE[key] = _build_nc(reps, variant)
    return _NC_CACHE[key]


def _host_inputs(output13, output26, output52, anchors13, anchors26, anchors52, thresh):
    consts = np.zeros((96, 8), np.float32)
    amod = np.arange(96) % 3
    for hi, anc in enumerate((anchors13, anchors26, anchors52)):
        anc = np.asarray(anc, np.float32)
        consts[:, 2 * hi] = np.log(anc[:, 0] / 2.0)[amod]
        consts[:, 2 * hi + 1] = np.log(anc[:, 1] / 2.0)[amod]
    consts[:, 6] = np.float32(thresh)
    consts[:, 7] = np.where(np.arange(96) >= 48, 208.0, 0.0)
    eye = np.eye(96, dtype=np.float32)
    idents = np.stack([eye, -eye, 32 * eye, 16 * eye, 8 * eye])
    full = {"13": np.asarray(output13, np.float32),
            "26": np.asarray(output26, np.float32),
            "52": np.asarray(output52, np.float32)}
    # prepack: [B,45,H,W] -> (g, b, a, c, k, f) so each partition row's chunk
    # (15 roles x FC cells) is one contiguous DMA descriptor
    packed = {}
    for name, H, S, G, NCH in HEADS:
        HW = H * H
        FC = HW // (G * NCH)
        x = full[name].reshape(B, 3, 15, G, NCH, FC)  # (b, a, k, g, c, f)
        packed[name] = x.transpose(3, 0, 1, 4, 2, 5)  # (g, b, a, c, k, f)
    in_maps = []
    for cid in range(NCORES):
        mcore = {"consts": consts, "idents": idents}
        for name, H, S, G, NCH in HEADS:
            sl = packed[name][:, cid * BL:(cid + 1) * BL]
            mcore[f"in{name}"] = np.ascontiguousarray(sl)
        in_maps.append(mcore)
    return in_maps


def _assemble(results):
    boxes_parts = []
    mask_parts = []
    for name, H, S, G, NCH in HEADS:
        HW = H * H
        FC = HW // (G * NCH)
        # [G, BL, 3, NCH, 7, FC] per core, concat batch
        ob = np.concatenate([r[f"out{name}"] for r in results], axis=1)
        # (g, b, a, c, r, f) -> (b, g, c, f, a, r) so (g,c,f) flattens to hw
        arr = ob.transpose(1, 0, 3, 5, 2, 4).reshape(B, HW, 3, 7)
        # packed plane order: x1, y1, x2, y2, kind, mask, iou
        boxes_parts.append(arr[..., [6, 0, 1, 2, 3, 4]].reshape(-1, 6))
        mask_parts.append(arr[..., 5].reshape(-1) != 0)
    boxes = np.ascontiguousarray(np.concatenate(boxes_parts, 0))
    mask = np.concatenate(mask_parts, 0)
    return boxes, mask


def _run(trace=False, **inputs):
    nc = _get_nc()
    in_maps = _host_inputs(**inputs)
    res = run_bass_kernel_spmd(nc, in_maps, list(range(NCORES)), trace=trace)
    out = _assemble(res.results)
    return out, res


def kernel(**inputs):
    out, _ = _run(trace=False, **inputs)
    return out


def kernel_traced(**inputs):
    return _run(trace=True, **inputs)
